# revision 52
# baseline (speedup 1.0000x reference)
"""BezierDeformableAttention Trainium2 kernel.

Sharding: 8 cores = (batch b in 0..3) x (head-group hg in 0..1, 4 heads each).
Each core computes its head-group's deformable-attention contribution for its
batch, pushed through the (linear) output projections; the host sums the two
half-contributions per batch (residual/biases are assigned to the hg==0 core).
"""
import sys
import os

sys.path.insert(0, "/opt/trn_rl_repo")

import numpy as np
import ml_dtypes

import concourse.bass as bass
import concourse.mybir as mybir
import concourse.tile as tile
import concourse.bacc as bacc
from concourse.bass_utils import run_bass_kernel_spmd
from concourse.masks import make_identity
from concourse import library_config

F32 = mybir.dt.float32
BF16 = mybir.dt.bfloat16
I32 = mybir.dt.int32
I16 = mybir.dt.int16
AOP = mybir.AluOpType
AFT = mybir.ActivationFunctionType
BF = ml_dtypes.bfloat16

B, N, D = 4, 1024, 256
HEADS, PTS = 8, 4
NH = 4                     # heads per core
H, W = 200, 200
HW = H * W                 # 40000
HD = 32                    # head dim
KS = 10                    # bezier samples

# value table: per head quad-rows (y, xquad) of 128 bf16 elems (256B)
ROWS_PER_HEAD = H * (W // 4)          # 10000
ELEMS_PER_HEAD = HW * HD              # 1280000
VTAB_ELEMS = NH * ELEMS_PER_HEAD + 128

NC_CHUNKS = 8              # n chunks of 128
SHIFT = 256.0              # positivity shift for floor-via-trunc

# packed f32 constant tensor column layout (rows used vary per field)
CF = {
    "ctrlx": (4, 512, 1024),
    "ctrly": (4, 1536, 1024),
    "coeffT": (4, 0, KS),
    "pc": (1, 10, 6),
    "ones20x": (KS, 16, 64),
    "ones20y": (KS, 80, 64),
    "S16": (16, 144, 16),
    "R16": (16, 160, 128),
    "jv0": (128, 288, 1),
    "jv1": (128, 289, 1),
    "bval": (128, 290, 1),
    "qscale": (128, 291, 1),
    "bq0": (128, 292, 1),
    "bq1": (128, 293, 1),
    "bdo0": (128, 294, 1),
    "bdo1": (128, 295, 1),
    "bout0": (128, 296, 1),
    "bout1": (128, 297, 1),
    "boffp": (64, 298, 1),
    "battn": (16, 299, 1),
    "Woffp0": (128, 300, 64),
    "Woffp1": (128, 364, 64),
    "Wattn0": (128, 428, 16),
    "Wattn1": (128, 444, 16),
    "idxoff": (16, 460, 1),
}
CF_COLS = 2560


def _ap(base, extra_off, dims):
    return bass.AP(base.tensor, base.offset + extra_off, [list(base.ap[0])] + dims)


def _dram_ap(base, extra_off, dims):
    return bass.AP(base.tensor, base.offset + extra_off, dims)


def build_program():
    nc = bacc.Bacc("TRN2", target_bir_lowering=False, debug=False,
               enable_asserts=True, num_devices=8)

    def din(name, shape, dt):
        return nc.dram_tensor(name, shape, dt, kind="ExternalInput")

    bev = din("bev", [256, HW], BF16)
    qeT = din("qeT", [256, N], BF16)
    Wq = din("Wq", [256, 256], BF16)
    Wval = din("Wval", [256, 128], BF16)
    Wdo = din("Wdo", [128, 256], BF16)
    Wout = din("Wout", [256, 256], BF16)
    constf = din("constf", [128, CF_COLS], F32)
    constb = din("constb", [128, 132], BF16)

    outT = nc.dram_tensor("outT", [256, N], F32, kind="ExternalOutput")

    with tile.TileContext(nc) as tc:
        cp = tc.alloc_tile_pool(name="cp", bufs=1)
        tp = tc.alloc_tile_pool(name="tp", bufs=5)
        repp = tc.alloc_tile_pool(name="repp", bufs=2)
        bevp = tc.alloc_tile_pool(name="bevp", bufs=2)
        vstp = tc.alloc_tile_pool(name="vstp", bufs=2)
        gp = tc.alloc_tile_pool(name="gp", bufs=2)
        myp = tc.alloc_tile_pool(name="myp", bufs=1)
        trp = tc.alloc_tile_pool(name="trp", bufs=1)
        wtp = tc.alloc_tile_pool(name="wtp", bufs=4)
        vps = tc.alloc_tile_pool(name="vps", bufs=5, space="PSUM")
        sps = tc.alloc_tile_pool(name="sps", bufs=2, space="PSUM")
        drp = tc.alloc_tile_pool(name="drp", bufs=1, space="DRAM")

        def load(name_ap, shape, dt, pool=cp, tag=None):
            nm = tag or ("ld_" + name_ap.name)
            t = pool.tile(shape, dt, tag=nm, name=nm)
            nc.sync.dma_start(out=t[:], in_=name_ap[:])
            return t

        # ---------- constant loads ----------
        # Wval first on the SP queue (bev loads + vt writes follow there);
        # stage-B-critical loads on the DVE queue; late-needed weights on the
        # Pool queue, keeping HWDGE head-of-line clear for bev.
        Wval_sb = [cp.tile([128, 128], BF16, tag=f"wv{k}", name=f"wv{k}") for k in range(2)]
        for k in range(2):
            nc.sync.dma_start(out=Wval_sb[k][:], in_=Wval[k * 128:(k + 1) * 128, :])
        constf_sb = cp.tile([128, CF_COLS], F32, tag="constf", name="constf")
        nc.scalar.dma_start(out=constf_sb[:], in_=constf[:, :])
        qeT_sb = [cp.tile([128, N], BF16, tag=f"qeT{k}", name=f"qeT{k}") for k in range(2)]
        for k in range(2):
            nc.scalar.dma_start(out=qeT_sb[k][:], in_=qeT[k * 128:(k + 1) * 128, :])
        Wq_sb = [cp.tile([128, 256], BF16, tag=f"wq{k}", name=f"wq{k}") for k in range(2)]
        for k in range(2):
            nc.scalar.dma_start(out=Wq_sb[k][:], in_=Wq[k * 128:(k + 1) * 128, :])
        constb_sb = cp.tile([128, 132], BF16, tag="constb", name="constb")
        nc.scalar.dma_start(out=constb_sb[:], in_=constb[:, :])
        Wdo_sb = cp.tile([128, 256], BF16, tag="wdo", name="wdo")
        nc.scalar.dma_start(out=Wdo_sb[:], in_=Wdo[:, :])
        Wout_sb = [cp.tile([128, 256], BF16, tag=f"wu{k}", name=f"wu{k}") for k in range(2)]
        for k in range(2):
            nc.scalar.dma_start(out=Wout_sb[k][:], in_=Wout[k * 128:(k + 1) * 128, :])

        def cfs(name):
            rows, c0, ncols = CF[name]
            return constf_sb[0:rows, c0:c0 + ncols]

        ctrlx_sb = cfs("ctrlx")
        ctrly_sb = cfs("ctrly")
        pc_sb = cfs("pc")
        coeffT_sb = cfs("coeffT")
        ones20x_sb = cfs("ones20x")
        ones20y_sb = cfs("ones20y")
        boffp_sb = cfs("boffp")
        battn_sb = cfs("battn")
        S16_sb = cfs("S16")
        R16_sb = cfs("R16")
        jv0_sb = cfs("jv0")
        jv1_sb = cfs("jv1")
        bval_sb = cfs("bval")
        qscale_sb = cfs("qscale")
        bq_sb = [cfs("bq0"), cfs("bq1")]
        bdo_sb = [cfs("bdo0"), cfs("bdo1")]
        bout_sb = [cfs("bout0"), cfs("bout1")]
        Woffp_sb = [cfs("Woffp0"), cfs("Woffp1")]
        Wattn_sb = [cfs("Wattn0"), cfs("Wattn1")]
        ones4_sb = constb_sb[0:128, 0:4]
        BB4_sb = constb_sb[0:4, 4:132]

        ident_f = cp.tile([128, 128], F32, name="t3")
        make_identity(nc, ident_f[:])
        ident_b = cp.tile([128, 128], BF16, name="t4")
        nc.vector.tensor_copy(out=ident_b[:], in_=ident_f[:])

        vt = drp.tile([VTAB_ELEMS], BF16, name="t5")

        # ---------- stage B: bezier / query / attn / weights / indices ----------
        # pc broadcast + spans
        ones14 = tp.tile([1, 4], F32, tag="tmp", name="ones14")
        nc.vector.memset(ones14[:], 1.0)
        ps_pc = sps.tile([128, 512], F32, tag="sps", space="PSUM", name="ps_pc")
        nc.tensor.matmul(out=ps_pc[:4, :6], lhsT=ones14[:], rhs=pc_sb,
                         start=True, stop=True)
        pcb = tp.tile([4, 6], F32, tag="tmp", name="pcb")
        nc.vector.tensor_copy(out=pcb[:], in_=ps_pc[:4, :6])
        span = tp.tile([4, 2], F32, tag="tmp", name="tmp")
        nc.vector.tensor_tensor(out=span[:], in0=pcb[:, 3:5], in1=pcb[:, 0:2],
                                op=AOP.subtract)
        rspan = tp.tile([4, 2], F32, tag="tmp", name="tmp")
        nc.vector.reciprocal(out=rspan[:], in_=span[:])

        def norm_ctrl(src, dcol):
            t = tp.tile([4, N], F32, tag="tmp", name="tmp")
            nc.vector.tensor_scalar(out=t[:], in0=src, scalar1=pcb[:, dcol:dcol + 1],
                                    scalar2=None, op0=AOP.subtract)
            o = tp.tile([4, N], F32, tag="tmp", name="tmp")
            nc.vector.tensor_scalar(out=o[:], in0=t[:], scalar1=rspan[:, dcol:dcol + 1],
                                    scalar2=None, op0=AOP.mult)
            return o

        cxn = norm_ctrl(ctrlx_sb, 0)
        cyn = norm_ctrl(ctrly_sb, 1)

        def bez_clip(cn):
            o = tp.tile([KS, N], F32, tag="tmp", name="tmp")
            for nn in range(2):
                ps = sps.tile([128, 512], F32, tag="sps", space="PSUM", name="sps")
                nc.tensor.matmul(out=ps[:KS, :],
                                 lhsT=coeffT_sb, rhs=cn[:, nn * 512:(nn + 1) * 512],
                                 start=True, stop=True)
                nc.vector.tensor_scalar(out=o[:, nn * 512:(nn + 1) * 512],
                                        in0=ps[:KS, :], scalar1=0.01, scalar2=0.99,
                                        op0=AOP.max, op1=AOP.min)
            return o

        clx = bez_clip(cxn)
        cly = bez_clip(cyn)

        # query projection qT = (qe @ Wq)^T + bq
        qT_sb = []
        for m in range(2):
            q = cp.tile([128, N], F32, tag=f"qT{m}", name=f"qT{m}")
            for nn in range(2):
                ps = sps.tile([128, 512], F32, tag="sps", space="PSUM", name="sps")
                for k in range(2):
                    nc.tensor.matmul(out=ps[:, :],
                                     lhsT=Wq_sb[k][:, m * 128:(m + 1) * 128],
                                     rhs=qeT_sb[k][:, nn * 512:(nn + 1) * 512],
                                     start=(k == 0), stop=(k == 1))
                nc.scalar.activation(out=q[:, nn * 512:(nn + 1) * 512], in_=ps[:],
                                     func=AFT.Identity, bias=bq_sb[m])
            qT_sb.append(q)

        # sampling coords xyT = 200*ref + q@Woffp + boffp  (shifted by 255.5)
        xyT = tp.tile([48, N], F32, tag="tmp", name="xyT")
        for nn in range(2):
            ps_xy = sps.tile([128, 512], F32, tag="sps", space="PSUM", name="sps")
            mm_args = [(Woffp_sb[k], qT_sb[k][:, nn * 512:(nn + 1) * 512])
                       for k in range(2)]
            mm_args.append((ones20x_sb, clx[:, nn * 512:(nn + 1) * 512]))
            mm_args.append((ones20y_sb, cly[:, nn * 512:(nn + 1) * 512]))
            for i, (lhsT, rhs) in enumerate(mm_args):
                nc.tensor.matmul(out=ps_xy[:64, :], lhsT=lhsT, rhs=rhs,
                                 start=(i == 0), stop=(i == 3))
            nc.scalar.activation(out=xyT[:, nn * 512:(nn + 1) * 512], in_=ps_xy[:48, :],
                                 func=AFT.Identity, bias=boffp_sb[0:48, :])

        # attention softmax (over 4 pts per head)
        logits = tp.tile([16, N], F32, tag="tmp", name="tmp")
        for nn in range(2):
            ps_at = sps.tile([128, 512], F32, tag="sps", space="PSUM", name="sps")
            for k in range(2):
                nc.tensor.matmul(out=ps_at[:16, :],
                                 lhsT=Wattn_sb[k],
                                 rhs=qT_sb[k][:, nn * 512:(nn + 1) * 512],
                                 start=(k == 0), stop=(k == 1))
            nc.scalar.activation(out=logits[:, nn * 512:(nn + 1) * 512],
                                 in_=ps_at[:16, :], func=AFT.Identity,
                                 bias=battn_sb)
        Ee = tp.tile([16, N], F32, tag="tmp", name="Ee")
        nc.scalar.activation(out=Ee[:], in_=logits[:], func=AFT.Exp)
        Rr = tp.tile([16, N], F32, tag="tmp", name="tmp")
        for nn in range(2):
            ps_sm = sps.tile([128, 512], F32, tag="sps", space="PSUM", name="sps")
            nc.tensor.matmul(out=ps_sm[:16, :], lhsT=S16_sb,
                             rhs=Ee[:, nn * 512:(nn + 1) * 512], start=True, stop=True)
            nc.vector.reciprocal(out=Rr[:, nn * 512:(nn + 1) * 512], in_=ps_sm[:16, :])
        attn = tp.tile([16, N], F32, tag="tmp", name="attn")
        nc.vector.tensor_tensor(out=attn[:], in0=Ee[:], in1=Rr[:], op=AOP.mult)

        # floor / frac
        ti = tp.tile([48, N], I32, tag="tmp", name="ti")
        nc.vector.tensor_copy(out=ti[:], in_=xyT[:])
        f0r = tp.tile([48, N], F32, tag="tmp", name="f0r")
        nc.vector.tensor_copy(out=f0r[:], in_=ti[:])
        gtc = tp.tile([48, N], F32, tag="tmp", name="gtc")
        nc.vector.tensor_tensor(out=gtc[:], in0=f0r[:], in1=xyT[:], op=AOP.is_gt)
        f0 = tp.tile([48, N], F32, tag="f0", name="f0", bufs=1)
        nc.vector.tensor_tensor(out=f0[:], in0=f0r[:], in1=gtc[:], op=AOP.subtract)
        wfr = tp.tile([48, N], F32, tag="wfr", name="wfr", bufs=1)
        nc.vector.tensor_tensor(out=wfr[:], in0=xyT[:], in1=f0[:], op=AOP.subtract)
        f0x = f0[0:16, :]
        f0yc = tp.tile([16, N], F32, tag="f0yc", name="f0yc", bufs=1)
        nc.vector.tensor_copy(out=f0yc[:], in_=f0[32:48, :])
        f0y = f0yc[:]
        wx = wfr[0:16, :]
        wyc = tp.tile([16, N], F32, tag="wyc", name="wyc", bufs=1)
        nc.vector.tensor_copy(out=wyc[:], in_=wfr[32:48, :])
        wy = wyc[:]

        # x-side: quad index, window position, validity
        c01s = tp.tile([16, N], F32, tag="tmp", name="c01s")
        nc.vector.tensor_scalar(out=c01s[:], in0=f0x, scalar1=SHIFT, scalar2=SHIFT + 199,
                                op0=AOP.max, op1=AOP.min)
        vx0 = tp.tile([16, N], F32, tag="tmp", name="vx0")
        nc.vector.tensor_tensor(out=vx0[:], in0=c01s[:], in1=f0x, op=AOP.is_equal)
        mcl = tp.tile([16, N], F32, tag="tmp", name="tmp")
        nc.vector.tensor_scalar(out=mcl[:], in0=f0x, scalar1=SHIFT - 1, scalar2=SHIFT + 198,
                                op0=AOP.max, op1=AOP.min)
        vx1 = tp.tile([16, N], F32, tag="tmp", name="vx1")
        nc.vector.tensor_tensor(out=vx1[:], in0=mcl[:], in1=f0x, op=AOP.is_equal)
        tq = tp.tile([16, N], F32, tag="tmp", name="tmp")
        nc.vector.tensor_scalar(out=tq[:], in0=c01s[:], scalar1=0.25, scalar2=None,
                                op0=AOP.mult)
        q0i = tp.tile([16, N], I32, tag="tmp", name="tmp")
        nc.vector.tensor_copy(out=q0i[:], in_=tq[:])
        q0r = tp.tile([16, N], F32, tag="tmp", name="q0r")
        nc.vector.tensor_copy(out=q0r[:], in_=q0i[:])
        qgt = tp.tile([16, N], F32, tag="tmp", name="qgt")
        nc.vector.tensor_tensor(out=qgt[:], in0=q0r[:], in1=tq[:], op=AOP.is_gt)
        q0f = tp.tile([16, N], F32, tag="q0f", name="q0f", bufs=1)
        nc.vector.tensor_tensor(out=q0f[:], in0=q0r[:], in1=qgt[:], op=AOP.subtract)
        u0 = tp.tile([16, N], F32, tag="tmp", name="u0")
        nc.vector.scalar_tensor_tensor(out=u0[:], in0=q0f[:], scalar=-4.0, in1=f0x,
                                       op0=AOP.mult, op1=AOP.add)

        # y-side: rows (clamped) + validity
        ya = tp.tile([16, N], F32, tag="tmp", name="ya")
        nc.vector.tensor_scalar(out=ya[:], in0=f0y, scalar1=SHIFT - 1, scalar2=SHIFT + 199,
                                op0=AOP.max, op1=AOP.min)
        r0s = tp.tile([16, N], F32, tag="r0s", name="r0s", bufs=1)
        nc.vector.tensor_scalar(out=r0s[:], in0=ya[:], scalar1=SHIFT, scalar2=None,
                                op0=AOP.max)
        r1s = tp.tile([16, N], F32, tag="r1s", name="r1s", bufs=1)
        nc.vector.tensor_scalar(out=r1s[:], in0=ya[:], scalar1=1.0, scalar2=SHIFT + 199,
                                op0=AOP.add, op1=AOP.min)
        vy0 = tp.tile([16, N], F32, tag="tmp", name="vy0")
        nc.vector.tensor_tensor(out=vy0[:], in0=r0s[:], in1=f0y, op=AOP.is_equal)
        vy1 = tp.tile([16, N], F32, tag="tmp", name="vy1")
        nc.vector.scalar_tensor_tensor(out=vy1[:], in0=f0y, scalar=1.0, in1=r1s[:],
                                       op0=AOP.add, op1=AOP.is_equal)

        # bilinear factors (x factors also carry validity)
        nxt = tp.tile([16, N], F32, tag="tmp", name="tmp")
        nc.vector.tensor_scalar(out=nxt[:], in0=wx, scalar1=-1.0, scalar2=1.0,
                                op0=AOP.mult, op1=AOP.add)
        XF0 = tp.tile([16, N], F32, tag="XF0", name="XF0", bufs=1)
        nc.vector.tensor_tensor(out=XF0[:], in0=nxt[:], in1=vx0[:], op=AOP.mult)
        XF1 = tp.tile([16, N], F32, tag="XF1", name="XF1", bufs=1)
        nc.vector.tensor_tensor(out=XF1[:], in0=wx, in1=vx1[:], op=AOP.mult)
        nyt = tp.tile([16, N], F32, tag="tmp", name="tmp")
        nc.vector.tensor_scalar(out=nyt[:], in0=wy, scalar1=-1.0, scalar2=1.0,
                                op0=AOP.mult, op1=AOP.add)
        BYA = tp.tile([16, N], F32, tag="tmp", name="tmp")
        nc.vector.tensor_tensor(out=BYA[:], in0=nyt[:], in1=vy0[:], op=AOP.mult)
        BYAa = tp.tile([16, N], F32, tag="BYAa", name="BYAa", bufs=1)
        nc.vector.tensor_tensor(out=BYAa[:], in0=BYA[:], in1=attn[:], op=AOP.mult)
        BYB = tp.tile([16, N], F32, tag="tmp", name="tmp")
        nc.vector.tensor_tensor(out=BYB[:], in0=wy, in1=vy1[:], op=AOP.mult)
        BYBa = tp.tile([16, N], F32, tag="BYBa", name="BYBa", bufs=1)
        nc.vector.tensor_tensor(out=BYBa[:], in0=BYB[:], in1=attn[:], op=AOP.mult)

        prods = []
        for yf in (BYAa, BYBa):
            for xf in (XF0, XF1):
                pr = tp.tile([16, N], F32, tag="tmp", name="tmp")
                nc.vector.tensor_tensor(out=pr[:], in0=xf[:], in1=yf[:], op=AOP.mult)
                prods.append(pr)

        # replicate (h,pt) rows across 8 x-window slots: rows (h,pt,x8)
        def rep128(src, dst=None, pool=repp, tag="rep"):
            o = dst or pool.tile([128, N], BF16, tag=tag, name=tag)
            for nn in range(2):
                ps = sps.tile([128, 512], F32, tag="sps", space="PSUM", name="sps")
                nc.tensor.matmul(out=ps[:], lhsT=R16_sb,
                                 rhs=src[:, nn * 512:(nn + 1) * 512],
                                 start=True, stop=True)
                nc.scalar.activation(out=o[:, nn * 512:(nn + 1) * 512], in_=ps[:],
                                     func=AFT.Copy)
            return o

        U_rep = cp.tile([128, N], BF16, tag="U_rep", name="U_rep")
        rep128(u0, dst=U_rep)

        W_y = []
        for y in range(2):
            ra = rep128(prods[2 * y + 0])
            rb = rep128(prods[2 * y + 1])
            t1 = tp.tile([128, N], F32, tag="tmp", name="tmp")
            nc.vector.scalar_tensor_tensor(out=t1[:], in0=U_rep[:], scalar=jv1_sb,
                                           in1=ra[:], op0=AOP.is_equal, op1=AOP.mult)
            t2 = tp.tile([128, N], F32, tag="tmp", name="tmp")
            nc.vector.scalar_tensor_tensor(out=t2[:], in0=U_rep[:], scalar=jv0_sb,
                                           in1=rb[:], op0=AOP.is_equal, op1=AOP.mult)
            w = cp.tile([128, N], BF16, tag=f"W_y{y}", name=f"W_y{y}")
            nc.vector.tensor_tensor(out=w[:], in0=t1[:], in1=t2[:], op=AOP.add)
            W_y.append(w)

        # gather row indices: idx = (r - SHIFT)*50 + (q0f - 64) + 10000*(h%2)
        # (odd heads offset into the second table of their head-pair); rows (y,h,pt)
        idxT = trp.tile([48, N], F32, tag="trA", name="idxT")
        for y, rs in ((0, r0s), (1, r1s)):
            tt = tp.tile([16, N], F32, tag="tmp", name="tmp")
            nc.vector.scalar_tensor_tensor(out=tt[:], in0=rs[:], scalar=50.0,
                                           in1=q0f[:], op0=AOP.mult, op1=AOP.add)
            nc.vector.tensor_scalar(out=idxT[32 * y:32 * y + 16, :], in0=tt[:],
                                    scalar1=cfs("idxoff"), scalar2=None,
                                    op0=AOP.subtract)

        # transpose idx to n-partitions, int16; free order (nc, h, pt, y)
        idxM = cp.tile([128, 256], I16, tag="idxM", name="idxM")
        for ncx in range(NC_CHUNKS):
            pst = sps.tile([128, 512], F32, tag="sps", space="PSUM", name="sps")
            nc.tensor.transpose(out=pst[:, :48], in_=idxT[:, ncx * 128:(ncx + 1) * 128],
                                identity=ident_f[:48, :48])
            mo = _ap(idxM[:], ncx * 32, [[1, 2], [8, 4], [2, 4]])
            mi = _ap(pst[:], 0, [[32, 2], [4, 4], [1, 4]])
            nc.vector.tensor_copy(out=mo, in_=mi)

        # fold partitions into free (stg[a, b*256+f] = idxM[b*16+a, f])
        stg16 = tp.tile([16, 2048], I16, tag="tmp", name="stg16")
        for bb in range(8):
            nc.sync.dma_start(out=stg16[0:16, bb * 256:(bb + 1) * 256],
                              in_=idxM[16 * bb:16 * (bb + 1), :])
        # interleave (b, f) -> (f*8 + b) with one strided DVE copy
        idx16 = cp.tile([128, 2048], I16, tag="idx16", name="idx16")
        so = _ap(stg16[0:16, :], 0, [[256, 8], [1, 256]])
        do = _ap(idx16[0:16, :], 0, [[1, 8], [8, 256]])
        nc.vector.tensor_copy(out=do, in_=so)
        nc.sync.dma_start(out=idx16[16:32, :], in_=idx16[0:16, :])
        nc.sync.dma_start(out=idx16[32:64, :], in_=idx16[0:32, :])
        nc.sync.dma_start(out=idx16[64:128, :], in_=idx16[0:64, :])

        # per-chunk transposed weights
        Wt = [[None] * NC_CHUNKS, [None] * NC_CHUNKS]
        for ncx in range(NC_CHUNKS):
            for y in range(2):
                pst = sps.tile([128, 128], BF16, tag="spsb", space="PSUM", name="spsb", bufs=1)
                nc.tensor.transpose(out=pst[:, :128],
                                    in_=W_y[y][:, ncx * 128:(ncx + 1) * 128],
                                    identity=ident_b[:])
                wt = wtp.tile([128, 128], BF16, tag=f"wt{y}_{ncx}", name=f"wt{y}_{ncx}", bufs=1)
                nc.vector.tensor_copy(out=wt[:], in_=pst[:, :128])
                Wt[y][ncx] = wt

        # ---------- stage C: value projection (quad-pair packed) ----------
        # per 1024-hw sub-group: psum [128 rho, 1024] where rho = quad-pair
        # (8 hw) and free = (h4, Q2, xh4, hd32); copy shuffles (xh,hd) ->
        # (hd,xh) so DRAM rows become per-head [quad, hd*4+xh] with 512B
        # contiguous runs per (partition, head).
        groups = [(i * 1024, 1024) for i in range(39)]
        gi = 0
        for g0, cols in groups:
            bevt = []
            for k in range(2):
                bt = bevp.tile([128, 1024], BF16, tag=f"bev{k}", name=f"bev{k}", bufs=4)
                eng = nc.sync if k == 0 else nc.scalar
                eng.dma_start(out=bt[:, :cols],
                              in_=bev[k * 128:(k + 1) * 128, g0:g0 + cols])
                bevt.append(bt)
            for sub in range(cols // 1024):
                vstage = vstp.tile([128, 1024], BF16, tag="vstage", name="vstage",
                                   bufs=4)
                for bank in range(2):
                    psv = vps.tile([128, 512], F32, tag="vps", space="PSUM",
                                   name="vps")
                    n = 0
                    for k in range(2):
                        for h in (2 * bank, 2 * bank + 1):
                            for Q in range(2):
                                for xh in range(4):
                                    nc.tensor.matmul(
                                        out=psv[:, (h % 2) * 256 + Q * 128
                                                + xh * 32:(h % 2) * 256 + Q * 128
                                                + xh * 32 + 32],
                                        lhsT=_ap(bevt[k][:],
                                                 sub * 1024 + 4 * Q + xh,
                                                 [[8, 128]]),
                                        rhs=Wval_sb[k][:, h * 32:(h + 1) * 32],
                                        start=(n == 0), stop=(n == 31),
                                        skip_group_check=True)
                                    n += 1
                    co = _ap(vstage[:], bank * 512, [[128, 4], [1, 4], [4, 32]])
                    ci = _ap(psv[:], 0, [[128, 4], [32, 4], [1, 32]])
                    nc.scalar.activation(out=co, in_=ci, func=AFT.Copy)
                dst = _dram_ap(vt[:], (g0 + sub * 1024) * 32,
                               [[256, 128], [ELEMS_PER_HEAD, 4], [1, 256]])
                nc.sync.dma_start(out=dst, in_=vstage[:])
                gi += 1
        # tail block: hw rows 39936..40000 (M=64 -> 8 quad-pairs)
        bt = bevp.tile([128, 1024], BF16, tag="bev0", name="bev0", bufs=4)
        nc.sync.dma_start(out=bt[:, :64], in_=bev[0:128, 39936:40000])
        bt2 = bevp.tile([128, 1024], BF16, tag="bev1", name="bev1", bufs=4)
        nc.sync.dma_start(out=bt2[:, :64], in_=bev[128:256, 39936:40000])
        bevt = (bt, bt2)
        vstage = vstp.tile([128, 1024], BF16, tag="vstage", name="vstage", bufs=4)
        for bank in range(2):
            psv = vps.tile([128, 512], F32, tag="vps", space="PSUM", name="vps")
            n = 0
            for h in (2 * bank, 2 * bank + 1):
                for Q in range(2):
                    for xh in range(4):
                        for k in range(2):
                            nc.tensor.matmul(
                                out=psv[:8, (h % 2) * 256 + Q * 128
                                        + xh * 32:(h % 2) * 256 + Q * 128
                                        + xh * 32 + 32],
                                lhsT=_ap(bevt[k][:], 4 * Q + xh, [[8, 8]]),
                                rhs=Wval_sb[k][:, h * 32:(h + 1) * 32],
                                start=(n == 0), stop=(n == 31),
                                skip_group_check=True)
                            n += 1
            co = _ap(vstage[0:8, :], bank * 512, [[128, 4], [1, 4], [4, 32]])
            ci = _ap(psv[0:8, :], 0, [[128, 4], [32, 4], [1, 32]])
            nc.scalar.activation(out=co, in_=ci, func=AFT.Copy)
        dst = _dram_ap(vt[:], 39936 * 32,
                       [[256, 8], [ELEMS_PER_HEAD, 4], [1, 256]])
        nc.sync.dma_start(out=dst, in_=vstage[0:8, :])

        # ---------- stage D: gather + weighted combine ----------
        if os.environ.get("KERNEL_SIM_LIB"):
            nc.gpsimd.load_library(library_config.mlp)
        # bias-of-value correction, precomputed early (only needs W_y)
        sw_sb = tp.tile([4, N], BF16, tag="sw", name="sw", bufs=1)
        for nn in range(2):
            ps_sw = sps.tile([128, 512], F32, tag="sps", space="PSUM", name="sps")
            for y in range(2):
                nc.tensor.matmul(out=ps_sw[:4, :], lhsT=ones4_sb,
                                 rhs=W_y[y][:, nn * 512:(nn + 1) * 512],
                                 start=(y == 0), stop=(y == 1))
            nc.scalar.activation(out=sw_sb[:, nn * 512:(nn + 1) * 512],
                                 in_=ps_sw[:4, :], func=AFT.Copy)
        corr = cp.tile([128, N], BF16, tag="corr", name="corr")
        for nn in range(2):
            ps_swr = sps.tile([128, 512], F32, tag="sps", space="PSUM", name="sps")
            nc.tensor.matmul(out=ps_swr[:], lhsT=BB4_sb,
                             rhs=sw_sb[:, nn * 512:(nn + 1) * 512], start=True, stop=True)
            nc.vector.tensor_scalar(out=corr[:, nn * 512:(nn + 1) * 512],
                                    in0=ps_swr[:], scalar1=bval_sb,
                                    scalar2=None, op0=AOP.mult)

        outT_sb = cp.tile([128, N], BF16, tag="outT_sb", name="outT_sb")
        outT2 = cp.tile([128, N], BF16, tag="outT2", name="outT2")
        msdaBF = [cp.tile([128, N], BF16, tag=f"msda{m}", name=f"msda{m}")
                  for m in range(2)]

        def stage_e_half(nn):
            # runs as soon as outT_sb cols nn*512..+512 are final
            sl = slice(nn * 512, (nn + 1) * 512)
            nc.vector.tensor_tensor(out=outT2[:, sl], in0=corr[:, sl],
                                    in1=outT_sb[:, sl], op=AOP.add)
            for m in range(2):
                psm = sps.tile([128, 512], F32, tag="sps", space="PSUM", name="sps")
                nc.tensor.matmul(out=psm[:],
                                 lhsT=Wdo_sb[:, m * 128:(m + 1) * 128],
                                 rhs=outT2[:, sl], start=True, stop=True)
                m1 = tp.tile([128, 512], F32, tag="tmp", name="tmp")
                nc.scalar.activation(out=m1[:], in_=psm[:],
                                     func=AFT.Identity, bias=bdo_sb[m])
                nc.vector.scalar_tensor_tensor(
                    out=msdaBF[m][:, sl], in0=qT_sb[m][:, sl], scalar=qscale_sb,
                    in1=m1[:], op0=AOP.mult, op1=AOP.add)
            for m in range(2):
                psf = sps.tile([128, 512], F32, tag="sps", space="PSUM", name="sps")
                for k in range(2):
                    nc.tensor.matmul(out=psf[:],
                                     lhsT=Wout_sb[k][:, m * 128:(m + 1) * 128],
                                     rhs=msdaBF[k][:, sl],
                                     start=(k == 0), stop=(k == 1))
                obh = tp.tile([128, 512], F32, tag="tmp", name="tmp")
                nc.scalar.activation(out=obh[:], in_=psf[:],
                                     func=AFT.Identity, bias=bout_sb[m])
                nc.sync.dma_start(out=outT[m * 128:(m + 1) * 128, sl], in_=obh[:])

        for ncx in range(NC_CHUNKS):
            G = gp.tile([128, 8192], BF16, tag="G", name="G")
            for h in range(NH):
                in_ap = _dram_ap(vt[:], (h - h % 2) * ELEMS_PER_HEAD,
                                 [[128, 2 * ROWS_PER_HEAD], [1, 256]])
                nc.gpsimd.dma_gather(
                    out_ap=G[:, h * 2048:(h + 1) * 2048].rearrange(
                        "p (j e) -> p j e", j=8),
                    in_ap=in_ap,
                    idxs_ap=idx16[:, (ncx * 4 + h) * 64:(ncx * 4 + h + 1) * 64],
                    num_idxs=1024, num_idxs_reg=1024,
                    elem_size=256, elem_step=128)
            # G token layout is (Q2, hd32, xh4); weights slot j = Q*4+xh.
            # Packed-last APs keep DVE in 2x mode for the big ops.
            My = []
            for y in range(2):
                m = myp.tile([128, 4096], BF16, tag=f"my{y}", name=f"my{y}")
                for Q in range(2):
                    nc.vector.tensor_tensor(
                        out=_ap(m[:], Q * 128, [[256, 16], [4, 32], [1, 4]]),
                        in0=_ap(G[:], y * 256 + Q * 128,
                                [[512, 16], [4, 32], [1, 4]]),
                        in1=_ap(Wt[y][ncx][:], Q * 4, [[8, 16], [0, 32], [1, 4]]),
                        op=AOP.mult)
                My.append(m)
            # fold y-sum and Q-sum into three 2048-wide adds -> (hpt, hd, xh)
            t2a = trp.tile([128, 2048], BF16, tag="trB", name="tr2a")
            nc.vector.tensor_tensor(
                out=_ap(t2a[:], 0, [[128, 16], [4, 32], [1, 4]]),
                in0=_ap(My[0][:], 0, [[256, 16], [4, 32], [1, 4]]),
                in1=_ap(My[0][:], 128, [[256, 16], [4, 32], [1, 4]]), op=AOP.add)
            t2b = trp.tile([128, 2048], BF16, tag="trA", name="tr2b")
            nc.vector.tensor_tensor(
                out=_ap(t2b[:], 0, [[128, 16], [4, 32], [1, 4]]),
                in0=_ap(t2a[:], 0, [[128, 16], [4, 32], [1, 4]]),
                in1=_ap(My[1][:], 0, [[256, 16], [4, 32], [1, 4]]), op=AOP.add)
            ts2 = trp.tile([128, 2048], BF16, tag="trB", name="tr2")
            nc.vector.tensor_tensor(
                out=_ap(ts2[:], 0, [[128, 16], [4, 32], [1, 4]]),
                in0=_ap(t2b[:], 0, [[128, 16], [4, 32], [1, 4]]),
                in1=_ap(My[1][:], 128, [[256, 16], [4, 32], [1, 4]]), op=AOP.add)
            # x-pairs (u, u+2) -> (hpt, hd, u) [128, 1024]
            ts3 = trp.tile([128, 1024], BF16, tag="trA", name="tr3")
            nc.vector.tensor_tensor(
                out=_ap(ts3[:], 0, [[64, 16], [2, 32], [1, 2]]),
                in0=_ap(ts2[:], 0, [[128, 16], [4, 32], [1, 2]]),
                in1=_ap(ts2[:], 2, [[128, 16], [4, 32], [1, 2]]), op=AOP.add)
            # u-sum -> (hpt, hd) [128, 512]
            ts4 = trp.tile([128, 512], BF16, tag="trB", name="tr4")
            nc.vector.tensor_tensor(
                out=_ap(ts4[:], 0, [[32, 16], [1, 32]]),
                in0=_ap(ts3[:], 0, [[64, 16], [2, 32]]),
                in1=_ap(ts3[:], 1, [[64, 16], [2, 32]]), op=AOP.add)
            # pt-pairs -> (h, j2, hd) [128, 256]
            ts5 = trp.tile([128, 256], BF16, tag="trA", name="tr5")
            nc.vector.tensor_tensor(
                out=_ap(ts5[:], 0, [[64, 4], [32, 2], [1, 32]]),
                in0=_ap(ts4[:], 0, [[128, 4], [64, 2], [1, 32]]),
                in1=_ap(ts4[:], 32, [[128, 4], [64, 2], [1, 32]]), op=AOP.add)
            och = trp.tile([128, 128], BF16, tag="trB", name="och")
            nc.vector.tensor_tensor(
                out=_ap(och[:], 0, [[32, 4], [1, 32]]),
                in0=_ap(ts5[:], 0, [[64, 4], [1, 32]]),
                in1=_ap(ts5[:], 32, [[64, 4], [1, 32]]), op=AOP.add)
            pst = sps.tile([128, 128], BF16, tag="spsb", space="PSUM", name="spsb", bufs=1)
            nc.tensor.transpose(out=pst[:, :128], in_=och[:], identity=ident_b[:])
            nc.scalar.activation(out=outT_sb[:, ncx * 128:(ncx + 1) * 128],
                                 in_=pst[:, :128], func=AFT.Copy)
            if ncx == 3:
                stage_e_half(0)
            elif ncx == 7:
                stage_e_half(1)


        for _pool in (drp, sps, vps, wtp, trp, myp, gp, vstp, bevp, repp, tp, cp):
            _pool.release()

    nc.compile()
    return nc


# ------------------------------------------------------------------
# host side
# ------------------------------------------------------------------
_NC_CACHE = {}


def _get_nc():
    if "nc" not in _NC_CACHE:
        _NC_CACHE["nc"] = build_program()
    return _NC_CACHE["nc"]


def _put(cf, name, arr):
    rows, c0, ncols = CF[name]
    a = np.asarray(arr, np.float32).reshape(rows, ncols)
    cf[0:rows, c0:c0 + ncols] = a


def _consts():
    """Batch-independent parts of constb and constf field values."""
    if "c" in _NC_CACHE:
        return _NC_CACHE["c"]
    t = np.linspace(0.0, 1.0, KS, dtype=np.float64)
    u = 1.0 - t
    coeff = np.stack([u ** 3, 3 * u ** 2 * t, 3 * u * t ** 2, t ** 3], -1)  # (10,4)
    coeffT = coeff.T.astype(np.float32)
    ones20x = np.zeros((KS, 64), np.float32)
    ones20x[:, 0:16] = 20.0
    ones20y = np.zeros((KS, 64), np.float32)
    ones20y[:, 32:48] = 20.0
    S16 = (np.arange(16)[:, None] // 4 == np.arange(16)[None, :] // 4).astype(np.float32)
    R16 = np.zeros((16, 128), np.float32)
    for hp in range(16):
        R16[hp, hp * 8:(hp + 1) * 8] = 1.0
    jv1 = (np.arange(128) % 8).astype(np.float32)[:, None]
    jv0 = jv1 - 1.0
    idxoff = np.array([SHIFT * 50 + 64 - 10000.0 * ((hp // 4) % 2)
                       for hp in range(16)], np.float32)[:, None]
    constb = np.zeros((128, 132), BF)
    for r in range(128):
        constb[r, r // 32] = 1
    for h in range(4):
        constb[h, 4 + h * 32:4 + (h + 1) * 32] = 1
    _NC_CACHE["c"] = (coeffT, ones20x, ones20y, S16, R16, jv0, jv1, idxoff, constb)
    return _NC_CACHE["c"]


def kernel(**inputs):
    qe = np.asarray(inputs["query_embed"], np.float32)
    ctrl = np.asarray(inputs["ctrl_points"], np.float32)
    bev = np.asarray(inputs["bev_features"], np.float32)
    pc_range = np.asarray(inputs["pc_range"], np.float32)
    W_q = np.asarray(inputs["W_q"], np.float32)
    b_q = np.asarray(inputs["b_q"], np.float32)
    W_val = np.asarray(inputs["W_val"], np.float32)
    b_val = np.asarray(inputs["b_val"], np.float32)
    W_off = np.asarray(inputs["W_off"], np.float32)
    b_off = np.asarray(inputs["b_off"], np.float32)
    W_attn = np.asarray(inputs["W_attn"], np.float32)
    b_attn = np.asarray(inputs["b_attn"], np.float32)
    W_do = np.asarray(inputs["W_do"], np.float32)
    b_do = np.asarray(inputs["b_do"], np.float32)
    W_out = np.asarray(inputs["W_out"], np.float32)
    b_out = np.asarray(inputs["b_out"], np.float32)

    nc = _get_nc()

    in_maps = [core_inputs(inputs, c) for c in range(8)]
    res = run_bass_kernel_spmd(nc, in_maps, core_ids=list(range(8)))
    kernel._last_results = res
    out = np.zeros((B, N, D), np.float32)
    for b in range(B):
        acc = res.results[2 * b]["outT"] + res.results[2 * b + 1]["outT"]
        out[b] = acc.T
    return out


def core_inputs(inputs, c):
    qe = np.asarray(inputs["query_embed"], np.float32)
    ctrl = np.asarray(inputs["ctrl_points"], np.float32)
    bev = np.asarray(inputs["bev_features"], np.float32)
    pc_range = np.asarray(inputs["pc_range"], np.float32)
    W_q = np.asarray(inputs["W_q"], np.float32)
    b_q = np.asarray(inputs["b_q"], np.float32)
    W_val = np.asarray(inputs["W_val"], np.float32)
    b_val = np.asarray(inputs["b_val"], np.float32)
    W_off = np.asarray(inputs["W_off"], np.float32)
    b_off = np.asarray(inputs["b_off"], np.float32)
    W_attn = np.asarray(inputs["W_attn"], np.float32)
    b_attn = np.asarray(inputs["b_attn"], np.float32)
    W_do = np.asarray(inputs["W_do"], np.float32)
    b_do = np.asarray(inputs["b_do"], np.float32)
    W_out = np.asarray(inputs["W_out"], np.float32)
    b_out = np.asarray(inputs["b_out"], np.float32)
    (coeffT, ones20x, ones20y, S16, R16, jv0, jv1, idxoff, constb) = _consts()
    b, hg = c // 2, c % 2
    # offsets: cols (d, h, pt) -> global col (hg*4+h)*8 + pt*2 + d
    Woffp = np.zeros((256, 64), np.float32)
    boffp = np.zeros((64, 1), np.float32)
    for d in range(2):
        for h in range(4):
            for pt in range(4):
                gcol = (hg * 4 + h) * 8 + pt * 2 + d
                lcol = d * 32 + h * 4 + pt
                Woffp[:, lcol] = W_off[:, gcol]
                boffp[lcol, 0] = b_off[gcol] + (SHIFT - 0.5)
    Wattn_c = np.zeros((256, 16), np.float32)
    battn_c = np.zeros((16, 1), np.float32)
    for h in range(4):
        for pt in range(4):
            Wattn_c[:, h * 4 + pt] = W_attn[:, (hg * 4 + h) * 4 + pt]
            battn_c[h * 4 + pt, 0] = b_attn[(hg * 4 + h) * 4 + pt]
    first = (hg == 0)

    cf = np.zeros((128, CF_COLS), np.float32)
    _put(cf, "ctrlx", ctrl[b, :, :, 0].T)
    _put(cf, "ctrly", ctrl[b, :, :, 1].T)
    _put(cf, "coeffT", coeffT)
    _put(cf, "pc", pc_range[None, :])
    _put(cf, "ones20x", ones20x)
    _put(cf, "ones20y", ones20y)
    _put(cf, "S16", S16)
    _put(cf, "R16", R16)
    _put(cf, "jv0", jv0)
    _put(cf, "jv1", jv1)
    _put(cf, "bval", b_val[hg * 128:(hg + 1) * 128][:, None])
    _put(cf, "qscale", np.full((128, 1), 1.0 if first else 0.0, np.float32))
    _put(cf, "bq0", b_q[0:128, None])
    _put(cf, "bq1", b_q[128:256, None])
    bdo = (b_do if first else np.zeros_like(b_do))
    _put(cf, "bdo0", bdo[0:128, None])
    _put(cf, "bdo1", bdo[128:256, None])
    bout = (b_out if first else np.zeros_like(b_out))
    _put(cf, "bout0", bout[0:128, None])
    _put(cf, "bout1", bout[128:256, None])
    _put(cf, "boffp", boffp)
    _put(cf, "battn", battn_c)
    _put(cf, "Woffp0", Woffp[0:128, :])
    _put(cf, "Woffp1", Woffp[128:256, :])
    _put(cf, "Wattn0", Wattn_c[0:128, :])
    _put(cf, "Wattn1", Wattn_c[128:256, :])
    _put(cf, "idxoff", idxoff)

    return {
        "bev": np.ascontiguousarray(bev[b].reshape(256, HW)).astype(BF),
        "qeT": np.ascontiguousarray(qe[b].T).astype(BF),
        "Wq": W_q.astype(BF),
        "Wval": np.ascontiguousarray(W_val[:, hg * 128:(hg + 1) * 128]).astype(BF),
        "Wdo": np.ascontiguousarray(W_do[hg * 128:(hg + 1) * 128, :]).astype(BF),
        "Wout": W_out.astype(BF),
        "constf": cf,
        "constb": constb,
    }

